# revision 49
# baseline (speedup 1.0000x reference)
"""Trainium2 8-core kernel for tie-grouped gated attention.

Sharding: batch-parallel — core c owns batch c end to end (all 8 heads),
so there is NO collective at all: the tie-group coupling enters only
through the host-precomputed tie-group x-sum (qm = xsum @ (Wq*scale/tie)),
and the output projection is fully local since all heads live on the core.

Key tricks:
  - j-packing AND i-packing: only unmasked key positions j (padded to
    PJ=NJ*128 on the partition dim) and only unmasked query positions i
    (padded to PJI on the free dim) flow through the S/exp/PV stream.
    Masked-i outputs are uniform attention = mean_j v, appended as a
    mv-filled column block [PJI, PJI+N) that the host un-permutes.
  - softmax without max-subtraction: logits bounded; exp(S)*exp(bias)
    with exp(bias) packed on host (zeros in all padding => padded j rows
    and padded i cols contribute exactly 0).
  - denominator via a ones-column interleaved into vm (33-wide head
    blocks), accumulated by the same PV matmuls.
  - engine balance: exp+sigmoid on Act, E-mult/recip/copies on DVE,
    broadcast+u-mult on GpSimd, eb DMA dispatch on GpSimd's SWDGE so the
    SP queue never backs up.
All matmuls bf16 with fp32 PSUM accumulation.
"""

import os
import sys

sys.path.insert(0, "/opt/trn_rl_repo")

import numpy as np
import ml_dtypes

B, N, DIM, H, DH = 8, 1024, 256, 8, 32
INNER = H * DH
TIE = 4
NCORES = 8
BF16 = ml_dtypes.bfloat16

LAST_EXEC_NS = None
LAST_TRACE = None

_compiled = None
_compiled_key = None


def _build(NJ, PJI):
    """NJ: number of 128-row j chunks; PJI: packed-i width (mult of 32)."""
    import concourse.bacc as bacc
    import concourse.mybir as mybir
    from concourse.tile import TileContext

    f32 = mybir.dt.float32
    bf16 = mybir.dt.bfloat16
    Exp = mybir.ActivationFunctionType.Exp
    Sigmoid = mybir.ActivationFunctionType.Sigmoid
    mult = mybir.AluOpType.mult

    PJ = NJ * 128
    NW = PJI + N                     # packed-i block + (pad,) masked-i block
    MAIN = min(512, PJI)             # first i-chunk width
    REST = PJI - MAIN                # second i-chunk width (0 if PJI<=512)
    assert NJ * max(REST, 1) <= 512

    nc = bacc.Bacc("TRN2", target_bir_lowering=False, debug=False,
                   num_devices=NCORES)

    # ---- DRAM parameters (per core = per batch) ----
    xTp = nc.declare_dram_parameter("xTp", [128, 2 * PJ], bf16, isOutput=False)
    xsum = nc.declare_dram_parameter("xsum", [128, 2 * PJI], bf16, isOutput=False)
    xTo = nc.declare_dram_parameter("xTo", [128, 2 * NW], bf16, isOutput=False)
    xsumc = nc.declare_dram_parameter("xsumc", [128, 2], bf16, isOutput=False)
    fp8 = mybir.dt.float8e4
    ebp = nc.declare_dram_parameter("ebp", [H * NJ * 128, PJI], fp8,
                                    isOutput=False)
    wq = nc.declare_dram_parameter("wq", [128, 2 * INNER], bf16, isOutput=False)
    wk = nc.declare_dram_parameter("wk", [128, 2 * INNER], bf16, isOutput=False)
    wv = nc.declare_dram_parameter("wv", [128, 2 * INNER], bf16, isOutput=False)
    wg = nc.declare_dram_parameter("wg", [128, 2 * DIM], bf16, isOutput=False)
    wout = nc.declare_dram_parameter("wout", [128, 2 * DIM], bf16, isOutput=False)
    bg = nc.declare_dram_parameter("bg", [128, 2], f32, isOutput=False)
    out_ext = nc.declare_dram_parameter("out", [2 * 128, NW], bf16,
                                        isOutput=True)

    DEBUG = bool(int(os.environ.get("KERNEL_DEBUG", "0")))
    if DEBUG:
        dbg_k = nc.declare_dram_parameter("dbg_k", [2 * 128, PJ], bf16,
                                          isOutput=True)
        dbg_qm = nc.declare_dram_parameter("dbg_qm", [2 * 128, PJI], bf16,
                                           isOutput=True)
        dbg_vm = nc.declare_dram_parameter("dbg_vm", [NJ * 128, H * 33], bf16,
                                           isOutput=True)
        dbg_h = nc.declare_dram_parameter("dbg_h", [2 * 128, PJI], bf16,
                                          isOutput=True)
        dbg_g = nc.declare_dram_parameter("dbg_g", [2 * 128, NW], bf16,
                                          isOutput=True)
        dbg_E = nc.declare_dram_parameter("dbg_E", [128, PJI], bf16,
                                          isOutput=True)
        dbg_pv = nc.declare_dram_parameter("dbg_pv", [128, PJI], f32,
                                           isOutput=True)

    # i-chunks of a [?, NW] row for the tail matmuls
    def chunks(width, step=512):
        out, off = [], 0
        while off < width:
            w = min(step, width - off)
            out.append((off, w))
            off += w
        return out

    NWC = chunks(NW)

    with TileContext(nc) as tc, \
         tc.tile_pool(name="cpool", bufs=1) as cpool, \
         tc.tile_pool(name="epool", bufs=12) as epool, \
         tc.tile_pool(name="rpool", bufs=2) as rpool, \
         tc.tile_pool(name="ebpool", bufs=4) as ebpool, \
         tc.tile_pool(name="ps_a", bufs=4, space="PSUM") as ps_a, \
         tc.tile_pool(name="ps_pv", bufs=2, space="PSUM") as ps_pv, \
         tc.tile_pool(name="ps_m", bufs=2, space="PSUM") as ps_m:

        # ---- constant loads, chunked so they fan out across DMA queues ----
        def cload(name, param, shape, dt, splits=None):
            t = cpool.tile(shape, dt, name=name, tag=name)
            if splits is None:
                nc.sync.dma_start(out=t, in_=param[:, :])
            else:
                for off, w in splits:
                    nc.sync.dma_start(out=t[:, off:off + w],
                                      in_=param[:, off:off + w])
            return t

        def dc_splits(m):
            out = []
            for dc in range(2):
                for off, w in chunks(m):
                    out.append((dc * m + off, w))
            return out

        wsplit = [(0, INNER), (INNER, INNER)]
        # first k matmul needs wk + the leading xTp chunk of BOTH dc halves —
        # dispatch those before everything else on the SP queue.
        wk_sb = cload("wk_sb", wk, [128, 2 * INNER], bf16, wsplit)
        xTp_sb = cpool.tile([128, 2 * PJ], bf16, name="xTp_sb", tag="xTp_sb")
        xtp_splits = dc_splits(PJ)
        xtp_splits.sort(key=lambda s: s[0] % PJ)
        for off, w in xtp_splits:
            nc.sync.dma_start(out=xTp_sb[:, off:off + w],
                              in_=xTp[:, off:off + w])
        wq_sb = cload("wq_sb", wq, [128, 2 * INNER], bf16, wsplit)
        xsum_sb = cload("xsum_sb", xsum, [128, 2 * PJI], bf16, dc_splits(PJI))
        wv_sb = cload("wv_sb", wv, [128, 2 * INNER], bf16, wsplit)
        xsumc_sb = cload("xsumc_sb", xsumc, [128, 2], bf16)
        wg_sb = cpool.tile([128, 2 * DIM], bf16, name="wg_sb", tag="wg_sb")
        wout_sb = cpool.tile([128, 2 * DIM], bf16, name="wout_sb",
                             tag="wout_sb")
        bg_sb = cpool.tile([128, 2], f32, name="bg_sb", tag="bg_sb")
        xTo_sb = cpool.tile([128, 2 * NW], bf16, name="xTo_sb", tag="xTo_sb")
        for off, w in wsplit:
            nc.scalar.dma_start(out=wg_sb[:, off:off + w],
                                in_=wg[:, off:off + w])
            nc.scalar.dma_start(out=wout_sb[:, off:off + w],
                                in_=wout[:, off:off + w])
        nc.scalar.dma_start(out=bg_sb, in_=bg[:, :])
        for off, w in dc_splits(NW):
            nc.gpsimd.dma_start(out=xTo_sb[:, off:off + w],
                                in_=xTo[:, off:off + w])

        # eb tiles: one per head [128, NJ*PJI]; prefetched 2-3 heads ahead,
        # chunk dispatch alternating GpSimd / SP so no queue backs up.
        eb_tiles = {}

        def eb_prefetch(h):
            t = ebpool.tile([128, NJ * PJI], bf16, name=f"eb{h}", tag="eb")
            for jc in range(NJ):
                nc.gpsimd.dma_start(
                    out=t[:, jc * PJI:(jc + 1) * PJI],
                    in_=ebp[(h * NJ + jc) * 128:(h * NJ + jc + 1) * 128, :])
            eb_tiles[h] = t

        eb_prefetch(0)
        eb_prefetch(1)
        eb_prefetch(2)

        # ---- pre-phase: k, v(+ones), qm, mv ----
        # k_sb[oc]: [128(inner chunk), PJ] bf16
        k_sb = []
        for oc in range(2):
            t = cpool.tile([128, PJ], bf16, name=f"k_sb{oc}", tag=f"k_sb{oc}")
            for off, w in chunks(PJ):
                ps = ps_a.tile([128, w], f32, name=f"ps_k{oc}_{off}", tag="a")
                for dc in range(2):
                    nc.tensor.matmul(
                        ps,
                        lhsT=wk_sb[:, dc * INNER + oc * 128:
                                   dc * INNER + (oc + 1) * 128],
                        rhs=xTp_sb[:, dc * PJ + off: dc * PJ + off + w],
                        start=(dc == 0), stop=(dc == 1))
                nc.vector.tensor_copy(out=t[:, off:off + w], in_=ps)
            k_sb.append(t)

        # qm_sb[oc]: [128, PJI]
        qm_sb = []
        for oc in range(2):
            t = cpool.tile([128, PJI], bf16, name=f"qm_sb{oc}", tag=f"qm_sb{oc}")
            for off, w in chunks(PJI):
                ps = ps_a.tile([128, w], f32, name=f"ps_q{oc}_{off}", tag="a")
                for dc in range(2):
                    nc.tensor.matmul(
                        ps,
                        lhsT=wq_sb[:, dc * INNER + oc * 128:
                                   dc * INNER + (oc + 1) * 128],
                        rhs=xsum_sb[:, dc * PJI + off: dc * PJI + off + w],
                        start=(dc == 0), stop=(dc == 1))
                nc.vector.tensor_copy(out=t[:, off:off + w], in_=ps)
            qm_sb.append(t)

        # vm_sb[jc]: [128(j), 8*33] = per-head (32 v cols + ones col)
        vm_sb = []
        for jc in range(NJ):
            ps = ps_a.tile([128, INNER], f32, name=f"ps_v{jc}", tag="a")
            for dc in range(2):
                nc.tensor.matmul(
                    ps,
                    lhsT=xTp_sb[:, dc * PJ + jc * 128: dc * PJ + (jc + 1) * 128],
                    rhs=wv_sb[:, dc * INNER:(dc + 1) * INNER],
                    start=(dc == 0), stop=(dc == 1))
            t = cpool.tile([128, H * 33], bf16, name=f"vm_sb{jc}",
                           tag=f"vm_sb{jc}")
            nc.gpsimd.memset(t, 1.0)
            nc.vector.tensor_copy(
                out=t[:, :].rearrange("p (h w) -> p h w", h=H, w=33)[:, :, 0:32],
                in_=ps[:, :].rearrange("p (h w) -> p h w", h=H, w=32))
            vm_sb.append(t)

        # mv_sb[oc]: [128, 1] f32 = mean over ALL N positions of v
        mv_sb = []
        for oc in range(2):
            ps = ps_m.tile([128, 1], f32, name=f"ps_mv{oc}", tag="m")
            for dc in range(2):
                nc.tensor.matmul(
                    ps,
                    lhsT=wv_sb[:, dc * INNER + oc * 128:
                               dc * INNER + (oc + 1) * 128],
                    rhs=xsumc_sb[:, dc:dc + 1],
                    start=(dc == 0), stop=(dc == 1))
            t = cpool.tile([128, 1], f32, name=f"mv_sb{oc}", tag=f"mv_sb{oc}")
            nc.vector.tensor_scalar_mul(t, ps, 1.0 / N)
            mv_sb.append(t)

        # gates: z = Wg^T x staged through SBUF (zg) so the PE never waits on
        # an Act round-trip; all sigmoids run as ONE contiguous Act block
        # (single sigmoid table load) at h=5.
        g_sb, hg_sb, zg_sb = [], [], []
        for oc in range(2):
            g_sb.append(cpool.tile([128, NW], bf16, name=f"g_sb{oc}",
                                   tag=f"g_sb{oc}"))
            hg_sb.append(cpool.tile([128, NW], bf16, name=f"hg_sb{oc}",
                                    tag=f"hg_sb{oc}"))
            zg_sb.append(cpool.tile([128, NW], bf16, name=f"zg_sb{oc}",
                                    tag=f"zg_sb{oc}"))

        g_jobs = [(oc, off, w) for oc in range(2) for off, w in NWC]

        def emit_g_job(job):
            oc, off, w = job
            ps = ps_m.tile([128, w], f32, name=f"ps_g{oc}_{off}", tag="m")
            for dc in range(2):
                nc.tensor.matmul(
                    ps,
                    lhsT=wg_sb[:, dc * DIM + oc * 128: dc * DIM + (oc + 1) * 128],
                    rhs=xTo_sb[:, dc * NW + off: dc * NW + off + w],
                    start=(dc == 0), stop=(dc == 1))
            nc.vector.tensor_copy(out=zg_sb[oc][:, off:off + w], in_=ps)

        def emit_sigmoid_pair(i):
            for oc, off, w in g_jobs[2 * i:2 * i + 2]:
                nc.scalar.activation(g_sb[oc][:, off:off + w],
                                     zg_sb[oc][:, off:off + w], Sigmoid,
                                     bias=bg_sb[:, oc:oc + 1])

        # y chunks: psum -> bf16 sbuf -> DRAM, each DMA split in two so the
        # drain spreads across queues.
        def emit_y(oc, off, w, dma_engs):
            ps = ps_a.tile([128, w], f32, name=f"ps_y{oc}_{off}", tag="a")
            for dc in range(2):
                nc.tensor.matmul(
                    ps,
                    lhsT=wout_sb[:, dc * DIM + oc * 128:
                                 dc * DIM + (oc + 1) * 128],
                    rhs=hg_sb[dc][:, off:off + w],
                    start=(dc == 0), stop=(dc == 1))
            y = rpool.tile([128, w], bf16, name=f"y{oc}_{off}", tag="y")
            nc.vector.tensor_copy(out=y, in_=ps)
            h2 = w // 2
            dma_engs[0].dma_start(
                out=out_ext[oc * 128:(oc + 1) * 128, off:off + h2],
                in_=y[:, 0:h2])
            dma_engs[1].dma_start(
                out=out_ext[oc * 128:(oc + 1) * 128, off + h2:off + w],
                in_=y[:, h2:w])

        # masked-i fill: hg[:, PJI:NW] = g * mv, and its y chunks — these
        # depend only on g/mv, so they run during the stream, off the tail.
        def emit_fill_block():
            for oc in range(2):
                nc.vector.tensor_scalar_mul(
                    hg_sb[oc][:, PJI:NW], g_sb[oc][:, PJI:NW], mv_sb[oc])
            for oc in range(2):
                for off, w in chunks(N):
                    emit_y(oc, PJI + off, w, (nc.sync, nc.sync))

        # h_sb[oc]: [128, PJI] attention output (packed i), bf16
        h_sb = []
        for oc in range(2):
            t = cpool.tile([128, PJI], bf16, name=f"h_sb{oc}", tag=f"h_sb{oc}")
            h_sb.append(t)

        ones1 = cpool.tile([1, 32], bf16, name="ones1", tag="ones1")
        nc.gpsimd.memset(ones1, 1.0)

        # ---- stream: software-pipelined by one head ----
        state = {}  # head -> (psum_pv, E_main list, E_rest)

        def emit_S(h):
            """S matmuls + exp + eb-mult for head h."""
            oc, hs = h // 4, (h % 4) * 32
            eb = eb_tiles[h]
            pv = ps_pv.tile([97 if REST else 33, MAIN], f32,
                            name=f"pv{h}", tag="pv")
            Ems = []
            for jc in range(NJ):
                ps = ps_a.tile([128, MAIN], f32, name=f"ps_s{h}_{jc}", tag="a")
                nc.tensor.matmul(
                    ps,
                    lhsT=k_sb[oc][hs:hs + 32, jc * 128:(jc + 1) * 128],
                    rhs=qm_sb[oc][hs:hs + 32, 0:MAIN],
                    start=True, stop=True, tile_position=(hs, 0))
                eS = epool.tile([128, MAIN], bf16, name=f"eS{h}_{jc}", tag="eS")
                nc.scalar.activation(eS, ps, Exp)
                E = epool.tile([128, MAIN], bf16, name=f"E{h}_{jc}", tag="E")
                nc.vector.tensor_tensor(
                    out=E, in0=eS,
                    in1=eb[:, jc * PJI: jc * PJI + MAIN], op=mult)
                Ems.append(E)
            Er = None
            if REST:
                psr = ps_m.tile([128, NJ * REST], f32, name=f"ps_sr{h}",
                                tag="m")
                for jc in range(NJ):
                    nc.tensor.matmul(
                        psr[:, jc * REST:(jc + 1) * REST],
                        lhsT=k_sb[oc][hs:hs + 32, jc * 128:(jc + 1) * 128],
                        rhs=qm_sb[oc][hs:hs + 32, MAIN:PJI],
                        start=True, stop=True, skip_group_check=True,
                        tile_position=(hs, 0))
                eSr = epool.tile([128, NJ * REST], bf16, name=f"eSr{h}",
                                 tag="eSr")
                nc.scalar.activation(eSr, psr, Exp)
                Er = epool.tile([128, NJ * REST], bf16, name=f"Er{h}", tag="Er")
                nc.vector.tensor_tensor(
                    out=Er[:, :].rearrange("p (j w) -> p j w", j=NJ, w=REST),
                    in0=eSr[:, :].rearrange("p (j w) -> p j w", j=NJ, w=REST),
                    in1=eb[:, :].rearrange("p (j w) -> p j w", j=NJ, w=PJI)
                        [:, :, MAIN:PJI],
                    op=mult)
            state[h] = (pv, Ems, Er)

        def emit_PV(h):
            pv, Ems, Er = state[h]
            for jc in range(NJ):
                nc.tensor.matmul(
                    pv[0:33, :],
                    lhsT=vm_sb[jc][:, h * 33:(h + 1) * 33],
                    rhs=Ems[jc],
                    start=(jc == 0), stop=(jc == NJ - 1))
            if REST:
                for jc in range(NJ):
                    nc.tensor.matmul(
                        pv[64:97, 0:REST],
                        lhsT=vm_sb[jc][:, h * 33:(h + 1) * 33],
                        rhs=Er[:, jc * REST:(jc + 1) * REST],
                        start=(jc == 0), stop=(jc == NJ - 1))

        def emit_blend(h):
            pv, Ems_d, Er_d = state.pop(h)
            if DEBUG and h == 0:
                for jc in range(NJ):
                    nc.sync.dma_start(out=dbg_E[:, 0:MAIN], in_=Ems_d[jc]) \
                        if jc == 0 else None
                if REST:
                    nc.sync.dma_start(out=dbg_E[:, MAIN:PJI],
                                      in_=Er_d[:, 0:REST])
                pvc = rpool.tile([128, MAIN], f32, name="pvc", tag="pvc")
                nc.scalar.copy(pvc[0:33, :], pv[0:33, :])
                if REST:
                    nc.scalar.copy(pvc[64:97, 0:REST], pv[64:97, 0:REST])
                nc.sync.dma_start(out=dbg_pv[:, 0:MAIN], in_=pvc)
            oc, hs = h // 4, (h % 4) * 32
            # main and rest chains kept separate so the main-side blend can
            # start as soon as the main PV accumulation stops.
            dr = rpool.tile([1, PJI], f32, name=f"dr{h}", tag="dr")
            rr = rpool.tile([1, PJI], f32, name=f"rr{h}", tag="rr")
            Rb = rpool.tile([32, PJI], f32, name=f"Rb{h}", tag="Rb")
            nc.vector.tensor_copy(out=dr[:, 0:MAIN], in_=pv[32:33, 0:MAIN])
            nc.vector.reciprocal_approx_fast(out=rr[:, 0:MAIN],
                                             in_=dr[:, 0:MAIN])
            nc.gpsimd.partition_broadcast(Rb[:, 0:MAIN], rr[:, 0:MAIN])
            nc.vector.tensor_tensor(
                out=h_sb[oc][hs:hs + 32, 0:MAIN],
                in0=pv[0:32, 0:MAIN], in1=Rb[:, 0:MAIN], op=mult)
            if REST:
                nc.vector.tensor_copy(out=dr[:, MAIN:PJI],
                                      in_=pv[96:97, 0:REST])
                nc.vector.reciprocal_approx_fast(out=rr[:, MAIN:PJI],
                                                 in_=dr[:, MAIN:PJI])
                nc.gpsimd.partition_broadcast(Rb[:, MAIN:PJI],
                                              rr[:, MAIN:PJI])
                nc.vector.tensor_tensor(
                    out=h_sb[oc][hs:hs + 32, MAIN:PJI],
                    in0=pv[64:96, 0:REST], in1=Rb[:, MAIN:PJI], op=mult)

        # pipeline: S(h) | PV(h-1), blend(h-1); g jobs trickle in 2 per head,
        # the sigmoid block and fill block land where Act has slack.
        emit_S(0)
        for h in range(1, H):
            if h + 2 < H:
                eb_prefetch(h + 2)
            emit_S(h)
            emit_PV(h - 1)
            emit_blend(h - 1)
            if 1 <= h <= 4:
                emit_g_job(g_jobs[2 * h - 2])
                emit_g_job(g_jobs[2 * h - 1])
            if 2 <= h <= 5:
                emit_sigmoid_pair(h - 2)
            elif h == 6:
                emit_fill_block()
        emit_PV(H - 1)
        emit_blend(H - 1)

        if DEBUG:
            for oc in range(2):
                nc.sync.dma_start(out=dbg_k[oc * 128:(oc + 1) * 128, :],
                                  in_=k_sb[oc])
                nc.sync.dma_start(out=dbg_qm[oc * 128:(oc + 1) * 128, :],
                                  in_=qm_sb[oc])
                nc.sync.dma_start(out=dbg_h[oc * 128:(oc + 1) * 128, :],
                                  in_=h_sb[oc])
                nc.sync.dma_start(out=dbg_g[oc * 128:(oc + 1) * 128, :],
                                  in_=g_sb[oc])
            for jc in range(NJ):
                nc.sync.dma_start(out=dbg_vm[jc * 128:(jc + 1) * 128, :],
                                  in_=vm_sb[jc])

        # ---- tail: hg packed = h*g, then the packed y chunks only ----
        for oc in range(2):
            nc.vector.tensor_tensor(
                out=hg_sb[oc][:, 0:PJI], in0=h_sb[oc],
                in1=g_sb[oc][:, 0:PJI], op=mult)
        for oc in range(2):
            for off, w in chunks(PJI):
                emit_y(oc, off, w,
                       (nc.scalar, nc.sync) if oc == 0 else (nc.sync, nc.scalar))

    nc.compile()
    return nc


def _host_prep(x, mask, attn_bias, Wq, Wkv, Wout, Wg, bg, NJ, PJI):
    scale = DH ** -0.5
    PJ = NJ * 128
    NW = PJI + N

    def b16(a):
        return np.ascontiguousarray(a).astype(BF16)

    def dcpack(w):
        m = w.shape[1]
        return np.ascontiguousarray(
            w.reshape(2, 128, m).transpose(1, 0, 2).reshape(128, 2 * m))

    wq_p = b16(dcpack(Wq * (scale / TIE)))
    wk_p = b16(dcpack(Wkv[:, :INNER]))
    wv_p = b16(dcpack(Wkv[:, INNER:]))
    wg_p = b16(dcpack(Wg))
    wout_p = b16(dcpack(Wout))
    bg_p = np.ascontiguousarray(bg.reshape(2, 128).T).astype(np.float32)

    xsum_g = [x[g * TIE:(g + 1) * TIE].sum(0) for g in range(2)]  # [N, DIM]

    in_maps = []
    sels = []
    for c in range(NCORES):
        sel = np.where(mask[c])[0]
        n1 = len(sel)
        sels.append(sel)

        xp = np.zeros((DIM, PJ), np.float32)
        xp[:, :n1] = x[c, sel, :].T
        xs = np.zeros((DIM, PJI), np.float32)
        xs[:, :n1] = xsum_g[c // TIE][sel, :].T
        xo = np.zeros((DIM, NW), np.float32)
        xo[:, :n1] = x[c, sel, :].T
        xo[:, PJI:PJI + (N - n1)] = x[c, ~mask[c], :].T
        xsc = x[c].sum(0).reshape(2, 128).T  # [128, 2]

        eb = np.zeros((H * NJ * 128, PJI), np.float32)
        bias_c = attn_bias[0]                                # [H, N, N]
        for h in range(H):
            blk = np.exp(bias_c[h][np.ix_(sel, sel)].T)      # [j, i] packed
            eb[h * NJ * 128: h * NJ * 128 + n1, :n1] = blk

        in_maps.append({
            "xTp": b16(dcpack(xp)),
            "xsum": b16(dcpack(xs)),
            "xTo": b16(dcpack(xo)),
            "xsumc": b16(xsc),
            "ebp": np.ascontiguousarray(eb).astype(ml_dtypes.float8_e4m3fn),
            "wq": wq_p, "wk": wk_p, "wv": wv_p,
            "wg": wg_p, "wout": wout_p, "bg": bg_p,
        })
    return in_maps, sels


def kernel(x, mask, attn_bias, tie_dim, Wq, Wkv, Wout, bout, Wg, bg):
    global _compiled, _compiled_key, LAST_EXEC_NS, LAST_TRACE
    x = np.asarray(x, np.float32)
    mask_np = np.asarray(mask)
    attn_bias = np.asarray(attn_bias, np.float32)
    assert int(tie_dim) == TIE
    assert x.shape == (B, N, DIM) and mask_np.shape == (B, N)

    from concourse.bass_utils import run_bass_kernel_spmd

    n1s = mask_np.astype(np.int64).sum(axis=1)
    mx = int(n1s.max())
    NJ = max((mx + 127) // 128, 1)
    PJI = max(((mx + 31) // 32) * 32, 32)
    dbg = os.environ.get("KERNEL_DEBUG", "0")
    if _compiled is None or _compiled_key != (NJ, PJI, dbg):
        _compiled = _build(NJ, PJI)
        _compiled_key = (NJ, PJI, dbg)
    nc = _compiled

    in_maps, sels = _host_prep(
        x, mask_np, attn_bias,
        np.asarray(Wq, np.float32), np.asarray(Wkv, np.float32),
        np.asarray(Wout, np.float32), np.asarray(Wg, np.float32),
        np.asarray(bg, np.float32), NJ, PJI)

    trace = bool(int(os.environ.get("KERNEL_TRACE", "0")))
    res = run_bass_kernel_spmd(nc, in_maps, core_ids=list(range(NCORES)),
                               trace=trace)
    LAST_EXEC_NS = res.exec_time_ns
    LAST_TRACE = getattr(res, "profile_json", None)

    bout_f = np.asarray(bout, np.float32)
    y = np.empty((B, N, DIM), np.float32)
    for c in range(NCORES):
        o = np.asarray(res.results[c]["out"], np.float32)  # [256, NW]
        sel = sels[c]
        n1 = len(sel)
        y[c, sel, :] = o[:, :n1].T
        y[c, ~mask_np[c], :] = o[:, PJI:PJI + (N - n1)].T
    y += bout_f
    return y


# revision 53
# speedup vs baseline: 1.0454x; 1.0454x over previous
"""Trainium2 8-core kernel for tie-grouped gated attention.

Sharding: batch-parallel — core c owns batch c end to end (all 8 heads),
so there is NO collective at all: the tie-group coupling enters only
through the host-precomputed tie-group x-sum (qm = xsum @ (Wq*scale/tie)),
and the output projection is fully local since all heads live on the core.

Key tricks:
  - j-packing AND i-packing: only unmasked key positions j (padded to
    PJ=NJ*128 on the partition dim) and only unmasked query positions i
    (padded to PJI on the free dim) flow through the S/exp/PV stream.
    Masked-i outputs are uniform attention = mean_j v, appended as a
    mv-filled column block [PJI, PJI+N) that the host un-permutes.
  - softmax without max-subtraction: logits bounded; exp(S)*exp(bias)
    with exp(bias) packed on host (zeros in all padding => padded j rows
    and padded i cols contribute exactly 0).
  - denominator via a ones-column interleaved into vm (33-wide head
    blocks), accumulated by the same PV matmuls.
  - engine balance: exp+sigmoid on Act, E-mult/recip/copies on DVE,
    broadcast+u-mult on GpSimd, eb DMA dispatch on GpSimd's SWDGE so the
    SP queue never backs up.
All matmuls bf16 with fp32 PSUM accumulation.
"""

import os
import sys

sys.path.insert(0, "/opt/trn_rl_repo")

import numpy as np
import ml_dtypes

B, N, DIM, H, DH = 8, 1024, 256, 8, 32
INNER = H * DH
TIE = 4
NCORES = 8
BF16 = ml_dtypes.bfloat16

LAST_EXEC_NS = None
LAST_TRACE = None

_compiled = None
_compiled_key = None


def _build(NJ, PJI):
    """NJ: number of 128-row j chunks; PJI: packed-i width (mult of 32)."""
    import concourse.bacc as bacc
    import concourse.mybir as mybir
    from concourse.tile import TileContext

    f32 = mybir.dt.float32
    bf16 = mybir.dt.bfloat16
    Exp = mybir.ActivationFunctionType.Exp
    Sigmoid = mybir.ActivationFunctionType.Sigmoid
    mult = mybir.AluOpType.mult

    PJ = NJ * 128
    NW = PJI + N                     # packed-i block + (pad,) masked-i block
    MAIN = min(512, PJI)             # first i-chunk width
    REST = PJI - MAIN                # second i-chunk width (0 if PJI<=512)
    assert NJ * max(REST, 1) <= 512

    nc = bacc.Bacc("TRN2", target_bir_lowering=False, debug=False,
                   num_devices=NCORES)

    # ---- DRAM parameters (per core = per batch) ----
    xTp = nc.declare_dram_parameter("xTp", [128, 2 * PJ], bf16, isOutput=False)
    xsum = nc.declare_dram_parameter("xsum", [128, 2 * PJI], bf16, isOutput=False)
    xTo = nc.declare_dram_parameter("xTo", [128, 2 * NW], bf16, isOutput=False)
    xsumc = nc.declare_dram_parameter("xsumc", [128, 2], bf16, isOutput=False)
    fp8 = mybir.dt.float8e4
    ebp = nc.declare_dram_parameter("ebp", [H * NJ * 128, PJI], fp8,
                                    isOutput=False)
    wq = nc.declare_dram_parameter("wq", [128, 2 * INNER], bf16, isOutput=False)
    wk = nc.declare_dram_parameter("wk", [128, 2 * INNER], bf16, isOutput=False)
    wv = nc.declare_dram_parameter("wv", [128, 2 * INNER], bf16, isOutput=False)
    wg = nc.declare_dram_parameter("wg", [128, 2 * DIM], bf16, isOutput=False)
    wout = nc.declare_dram_parameter("wout", [128, 2 * DIM], bf16, isOutput=False)
    bg = nc.declare_dram_parameter("bg", [128, 2], f32, isOutput=False)
    out_ext = nc.declare_dram_parameter("out", [2 * 128, NW], bf16,
                                        isOutput=True)

    DEBUG = bool(int(os.environ.get("KERNEL_DEBUG", "0")))
    if DEBUG:
        dbg_k = nc.declare_dram_parameter("dbg_k", [2 * 128, PJ], bf16,
                                          isOutput=True)
        dbg_qm = nc.declare_dram_parameter("dbg_qm", [2 * 128, PJI], bf16,
                                           isOutput=True)
        dbg_vm = nc.declare_dram_parameter("dbg_vm", [NJ * 128, H * 33], bf16,
                                           isOutput=True)
        dbg_h = nc.declare_dram_parameter("dbg_h", [2 * 128, PJI], bf16,
                                          isOutput=True)
        dbg_g = nc.declare_dram_parameter("dbg_g", [2 * 128, NW], bf16,
                                          isOutput=True)
        dbg_E = nc.declare_dram_parameter("dbg_E", [128, PJI], bf16,
                                          isOutput=True)
        dbg_pv = nc.declare_dram_parameter("dbg_pv", [128, PJI], f32,
                                           isOutput=True)

    # i-chunks of a [?, NW] row for the tail matmuls
    def chunks(width, step=512):
        out, off = [], 0
        while off < width:
            w = min(step, width - off)
            out.append((off, w))
            off += w
        return out

    NWC = chunks(NW)

    with TileContext(nc) as tc, \
         tc.tile_pool(name="cpool", bufs=1) as cpool, \
         tc.tile_pool(name="epool", bufs=12) as epool, \
         tc.tile_pool(name="rpool", bufs=2) as rpool, \
         tc.tile_pool(name="ebpool", bufs=1) as ebpool, \
         tc.tile_pool(name="ps_a", bufs=4, space="PSUM") as ps_a, \
         tc.tile_pool(name="ps_pv", bufs=2, space="PSUM") as ps_pv, \
         tc.tile_pool(name="ps_m", bufs=2, space="PSUM") as ps_m:

        # ---- constant loads, chunked so they fan out across DMA queues ----
        def cload(name, param, shape, dt, splits=None):
            t = cpool.tile(shape, dt, name=name, tag=name)
            if splits is None:
                nc.sync.dma_start(out=t, in_=param[:, :])
            else:
                for off, w in splits:
                    nc.sync.dma_start(out=t[:, off:off + w],
                                      in_=param[:, off:off + w])
            return t

        def dc_splits(m):
            out = []
            for dc in range(2):
                for off, w in chunks(m):
                    out.append((dc * m + off, w))
            return out

        wsplit = [(0, INNER), (INNER, INNER)]
        # first k matmul needs wk + the leading xTp chunk of BOTH dc halves —
        # dispatch those before everything else on the SP queue.
        wk_sb = cload("wk_sb", wk, [128, 2 * INNER], bf16, wsplit)
        xTp_sb = cpool.tile([128, 2 * PJ], bf16, name="xTp_sb", tag="xTp_sb")
        xtp_splits = dc_splits(PJ)
        xtp_splits.sort(key=lambda s: s[0] % PJ)
        for off, w in xtp_splits:
            nc.sync.dma_start(out=xTp_sb[:, off:off + w],
                              in_=xTp[:, off:off + w])
        wq_sb = cload("wq_sb", wq, [128, 2 * INNER], bf16, wsplit)
        xsum_sb = cload("xsum_sb", xsum, [128, 2 * PJI], bf16, dc_splits(PJI))
        wv_sb = cload("wv_sb", wv, [128, 2 * INNER], bf16, wsplit)
        xsumc_sb = cload("xsumc_sb", xsumc, [128, 2], bf16)
        wg_sb = cpool.tile([128, 2 * DIM], bf16, name="wg_sb", tag="wg_sb")
        wout_sb = cpool.tile([128, 2 * DIM], bf16, name="wout_sb",
                             tag="wout_sb")
        bg_sb = cpool.tile([128, 2], f32, name="bg_sb", tag="bg_sb")
        xTo_sb = cpool.tile([128, 2 * NW], bf16, name="xTo_sb", tag="xTo_sb")
        for off, w in wsplit:
            nc.scalar.dma_start(out=wg_sb[:, off:off + w],
                                in_=wg[:, off:off + w])
            nc.scalar.dma_start(out=wout_sb[:, off:off + w],
                                in_=wout[:, off:off + w])
        nc.scalar.dma_start(out=bg_sb, in_=bg[:, :])
        for off, w in dc_splits(NW):
            nc.gpsimd.dma_start(out=xTo_sb[:, off:off + w],
                                in_=xTo[:, off:off + w])

        # eb: fp8 exp-bias for ALL 8 heads resident in SBUF (22KB/partition);
        # loaded once at startup, chunks spread over the SP and GpSimd queues.
        eb_tiles = {}
        for h in range(H):
            t = ebpool.tile([128, NJ * PJI], fp8, name=f"eb{h}", tag=f"eb{h}")
            for jc in range(NJ):
                eng = nc.sync if (h * NJ + jc) % 2 == 0 else nc.gpsimd
                eng.dma_start(
                    out=t[:, jc * PJI:(jc + 1) * PJI],
                    in_=ebp[(h * NJ + jc) * 128:(h * NJ + jc + 1) * 128, :])
            eb_tiles[h] = t

        # ---- pre-phase: k, v(+ones), qm, mv ----
        # k_sb[oc]: [128(inner chunk), PJ] bf16
        k_sb = []
        for oc in range(2):
            t = cpool.tile([128, PJ], bf16, name=f"k_sb{oc}", tag=f"k_sb{oc}")
            for off, w in chunks(PJ):
                ps = ps_a.tile([128, w], f32, name=f"ps_k{oc}_{off}", tag="a")
                for dc in range(2):
                    nc.tensor.matmul(
                        ps,
                        lhsT=wk_sb[:, dc * INNER + oc * 128:
                                   dc * INNER + (oc + 1) * 128],
                        rhs=xTp_sb[:, dc * PJ + off: dc * PJ + off + w],
                        start=(dc == 0), stop=(dc == 1))
                nc.vector.tensor_copy(out=t[:, off:off + w], in_=ps)
            k_sb.append(t)

        # qm_sb[oc]: [128, PJI]
        qm_sb = []
        for oc in range(2):
            t = cpool.tile([128, PJI], bf16, name=f"qm_sb{oc}", tag=f"qm_sb{oc}")
            for off, w in chunks(PJI):
                ps = ps_a.tile([128, w], f32, name=f"ps_q{oc}_{off}", tag="a")
                for dc in range(2):
                    nc.tensor.matmul(
                        ps,
                        lhsT=wq_sb[:, dc * INNER + oc * 128:
                                   dc * INNER + (oc + 1) * 128],
                        rhs=xsum_sb[:, dc * PJI + off: dc * PJI + off + w],
                        start=(dc == 0), stop=(dc == 1))
                nc.vector.tensor_copy(out=t[:, off:off + w], in_=ps)
            qm_sb.append(t)

        # vm_sb[jc]: [128(j), 8*33] = per-head (32 v cols + ones col)
        vm_sb = []
        for jc in range(NJ):
            ps = ps_a.tile([128, INNER], f32, name=f"ps_v{jc}", tag="a")
            for dc in range(2):
                nc.tensor.matmul(
                    ps,
                    lhsT=xTp_sb[:, dc * PJ + jc * 128: dc * PJ + (jc + 1) * 128],
                    rhs=wv_sb[:, dc * INNER:(dc + 1) * INNER],
                    start=(dc == 0), stop=(dc == 1))
            t = cpool.tile([128, H * 33], bf16, name=f"vm_sb{jc}",
                           tag=f"vm_sb{jc}")
            nc.gpsimd.memset(t, 1.0)
            nc.vector.tensor_copy(
                out=t[:, :].rearrange("p (h w) -> p h w", h=H, w=33)[:, :, 0:32],
                in_=ps[:, :].rearrange("p (h w) -> p h w", h=H, w=32))
            vm_sb.append(t)

        # mv_sb[oc]: [128, 1] f32 = mean over ALL N positions of v
        mv_sb = []
        for oc in range(2):
            ps = ps_m.tile([128, 1], f32, name=f"ps_mv{oc}", tag="m")
            for dc in range(2):
                nc.tensor.matmul(
                    ps,
                    lhsT=wv_sb[:, dc * INNER + oc * 128:
                               dc * INNER + (oc + 1) * 128],
                    rhs=xsumc_sb[:, dc:dc + 1],
                    start=(dc == 0), stop=(dc == 1))
            t = cpool.tile([128, 1], f32, name=f"mv_sb{oc}", tag=f"mv_sb{oc}")
            nc.vector.tensor_scalar_mul(t, ps, 1.0 / N)
            mv_sb.append(t)

        # gates: z = Wg^T x staged through SBUF (zg) so the PE never waits on
        # an Act round-trip; all sigmoids run as ONE contiguous Act block
        # (single sigmoid table load) at h=5.
        g_sb, hg_sb, zg_sb = [], [], []
        for oc in range(2):
            g_sb.append(cpool.tile([128, NW], bf16, name=f"g_sb{oc}",
                                   tag=f"g_sb{oc}"))
            hg_sb.append(cpool.tile([128, NW], bf16, name=f"hg_sb{oc}",
                                    tag=f"hg_sb{oc}"))
            zg_sb.append(cpool.tile([128, NW], bf16, name=f"zg_sb{oc}",
                                    tag=f"zg_sb{oc}"))

        g_jobs = [(oc, off, w) for oc in range(2) for off, w in NWC]

        def emit_g_job(job):
            oc, off, w = job
            ps = ps_m.tile([128, w], f32, name=f"ps_g{oc}_{off}", tag="m")
            for dc in range(2):
                nc.tensor.matmul(
                    ps,
                    lhsT=wg_sb[:, dc * DIM + oc * 128: dc * DIM + (oc + 1) * 128],
                    rhs=xTo_sb[:, dc * NW + off: dc * NW + off + w],
                    start=(dc == 0), stop=(dc == 1))
            nc.vector.tensor_copy(out=zg_sb[oc][:, off:off + w], in_=ps)

        def emit_sigmoid_block():
            for oc, off, w in g_jobs:
                nc.scalar.activation(g_sb[oc][:, off:off + w],
                                     zg_sb[oc][:, off:off + w], Sigmoid,
                                     bias=bg_sb[:, oc:oc + 1])

        # y chunks: psum -> bf16 sbuf -> DRAM, each DMA split in two so the
        # drain spreads across queues.
        def emit_y(oc, off, w, dma_engs):
            ps = ps_a.tile([128, w], f32, name=f"ps_y{oc}_{off}", tag="a")
            for dc in range(2):
                nc.tensor.matmul(
                    ps,
                    lhsT=wout_sb[:, dc * DIM + oc * 128:
                                 dc * DIM + (oc + 1) * 128],
                    rhs=hg_sb[dc][:, off:off + w],
                    start=(dc == 0), stop=(dc == 1))
            y = rpool.tile([128, w], bf16, name=f"y{oc}_{off}", tag="y")
            nc.vector.tensor_copy(out=y, in_=ps)
            h2 = w // 2
            dma_engs[0].dma_start(
                out=out_ext[oc * 128:(oc + 1) * 128, off:off + h2],
                in_=y[:, 0:h2])
            dma_engs[1].dma_start(
                out=out_ext[oc * 128:(oc + 1) * 128, off + h2:off + w],
                in_=y[:, h2:w])

        # masked-i fill: hg[:, PJI:NW] = g * mv, and its y chunks — these
        # depend only on g/mv, so they run during the stream, off the tail.
        def emit_fill_block():
            for oc in range(2):
                nc.vector.tensor_scalar_mul(
                    hg_sb[oc][:, PJI:NW], g_sb[oc][:, PJI:NW], mv_sb[oc])
            for oc in range(2):
                for off, w in chunks(N):
                    emit_y(oc, PJI + off, w, (nc.sync, nc.sync))

        # h_sb[oc]: [128, PJI] attention output (packed i), bf16
        h_sb = []
        for oc in range(2):
            t = cpool.tile([128, PJI], bf16, name=f"h_sb{oc}", tag=f"h_sb{oc}")
            h_sb.append(t)

        ones1 = cpool.tile([1, 32], bf16, name="ones1", tag="ones1")
        nc.gpsimd.memset(ones1, 1.0)

        # ---- stream: software-pipelined by one head ----
        state = {}  # head -> (psum_pv, E_main list, E_rest)

        def emit_S(h):
            """S matmuls + exp + eb-mult for head h."""
            oc, hs = h // 4, (h % 4) * 32
            eb = eb_tiles[h]
            pv = ps_pv.tile([97 if REST else 33, MAIN], f32,
                            name=f"pv{h}", tag="pv")
            Ems = []
            for jc in range(NJ):
                ps = ps_a.tile([128, MAIN], f32, name=f"ps_s{h}_{jc}", tag="a")
                nc.tensor.matmul(
                    ps,
                    lhsT=k_sb[oc][hs:hs + 32, jc * 128:(jc + 1) * 128],
                    rhs=qm_sb[oc][hs:hs + 32, 0:MAIN],
                    start=True, stop=True, tile_position=(hs, 0))
                eS = epool.tile([128, MAIN], bf16, name=f"eS{h}_{jc}", tag="eS")
                nc.scalar.activation(eS, ps, Exp)
                E = epool.tile([128, MAIN], bf16, name=f"E{h}_{jc}", tag="E")
                nc.vector.tensor_tensor(
                    out=E, in0=eS,
                    in1=eb[:, jc * PJI: jc * PJI + MAIN], op=mult)
                Ems.append(E)
            Er = None
            if REST:
                psr = ps_m.tile([128, NJ * REST], f32, name=f"ps_sr{h}",
                                tag="m")
                for jc in range(NJ):
                    nc.tensor.matmul(
                        psr[:, jc * REST:(jc + 1) * REST],
                        lhsT=k_sb[oc][hs:hs + 32, jc * 128:(jc + 1) * 128],
                        rhs=qm_sb[oc][hs:hs + 32, MAIN:PJI],
                        start=True, stop=True, skip_group_check=True,
                        tile_position=(hs, 0))
                eSr = epool.tile([128, NJ * REST], bf16, name=f"eSr{h}",
                                 tag="eSr")
                nc.scalar.activation(eSr, psr, Exp)
                Er = epool.tile([128, NJ * REST], bf16, name=f"Er{h}", tag="Er")
                nc.vector.tensor_tensor(
                    out=Er[:, :].rearrange("p (j w) -> p j w", j=NJ, w=REST),
                    in0=eSr[:, :].rearrange("p (j w) -> p j w", j=NJ, w=REST),
                    in1=eb[:, :].rearrange("p (j w) -> p j w", j=NJ, w=PJI)
                        [:, :, MAIN:PJI],
                    op=mult)
            state[h] = (pv, Ems, Er)

        def emit_PV(h):
            pv, Ems, Er = state[h]
            for jc in range(NJ):
                nc.tensor.matmul(
                    pv[0:33, :],
                    lhsT=vm_sb[jc][:, h * 33:(h + 1) * 33],
                    rhs=Ems[jc],
                    start=(jc == 0), stop=(jc == NJ - 1))
            if REST:
                for jc in range(NJ):
                    nc.tensor.matmul(
                        pv[64:97, 0:REST],
                        lhsT=vm_sb[jc][:, h * 33:(h + 1) * 33],
                        rhs=Er[:, jc * REST:(jc + 1) * REST],
                        start=(jc == 0), stop=(jc == NJ - 1))

        def emit_blend(h):
            pv, Ems_d, Er_d = state.pop(h)
            if DEBUG and h == 0:
                for jc in range(NJ):
                    nc.sync.dma_start(out=dbg_E[:, 0:MAIN], in_=Ems_d[jc]) \
                        if jc == 0 else None
                if REST:
                    nc.sync.dma_start(out=dbg_E[:, MAIN:PJI],
                                      in_=Er_d[:, 0:REST])
                pvc = rpool.tile([128, MAIN], f32, name="pvc", tag="pvc")
                nc.scalar.copy(pvc[0:33, :], pv[0:33, :])
                if REST:
                    nc.scalar.copy(pvc[64:97, 0:REST], pv[64:97, 0:REST])
                nc.sync.dma_start(out=dbg_pv[:, 0:MAIN], in_=pvc)
            oc, hs = h // 4, (h % 4) * 32
            # main and rest chains kept separate so the main-side blend can
            # start as soon as the main PV accumulation stops.
            dr = rpool.tile([1, PJI], f32, name=f"dr{h}", tag="dr")
            rr = rpool.tile([1, PJI], f32, name=f"rr{h}", tag="rr")
            Rb = rpool.tile([32, PJI], f32, name=f"Rb{h}", tag="Rb")
            nc.vector.tensor_copy(out=dr[:, 0:MAIN], in_=pv[32:33, 0:MAIN])
            nc.vector.reciprocal_approx_fast(out=rr[:, 0:MAIN],
                                             in_=dr[:, 0:MAIN])
            nc.gpsimd.partition_broadcast(Rb[:, 0:MAIN], rr[:, 0:MAIN])
            nc.vector.tensor_tensor(
                out=h_sb[oc][hs:hs + 32, 0:MAIN],
                in0=pv[0:32, 0:MAIN], in1=Rb[:, 0:MAIN], op=mult)
            if REST:
                nc.vector.tensor_copy(out=dr[:, MAIN:PJI],
                                      in_=pv[96:97, 0:REST])
                nc.vector.reciprocal_approx_fast(out=rr[:, MAIN:PJI],
                                                 in_=dr[:, MAIN:PJI])
                nc.gpsimd.partition_broadcast(Rb[:, MAIN:PJI],
                                              rr[:, MAIN:PJI])
                nc.vector.tensor_tensor(
                    out=h_sb[oc][hs:hs + 32, MAIN:PJI],
                    in0=pv[64:96, 0:REST], in1=Rb[:, MAIN:PJI], op=mult)

        # pipeline: S(h) | PV(h-1), blend(h-1); g jobs trickle in 2 per head,
        # the sigmoid block and fill block land where Act has slack.
        emit_S(0)
        for h in range(1, H):
            emit_S(h)
            emit_PV(h - 1)
            emit_blend(h - 1)
            if 1 <= h <= 4:
                emit_g_job(g_jobs[2 * h - 2])
                emit_g_job(g_jobs[2 * h - 1])
            if h == 5:
                emit_sigmoid_block()
            elif h == 6:
                emit_fill_block()
        emit_PV(H - 1)
        emit_blend(H - 1)

        if DEBUG:
            for oc in range(2):
                nc.sync.dma_start(out=dbg_k[oc * 128:(oc + 1) * 128, :],
                                  in_=k_sb[oc])
                nc.sync.dma_start(out=dbg_qm[oc * 128:(oc + 1) * 128, :],
                                  in_=qm_sb[oc])
                nc.sync.dma_start(out=dbg_h[oc * 128:(oc + 1) * 128, :],
                                  in_=h_sb[oc])
                nc.sync.dma_start(out=dbg_g[oc * 128:(oc + 1) * 128, :],
                                  in_=g_sb[oc])
            for jc in range(NJ):
                nc.sync.dma_start(out=dbg_vm[jc * 128:(jc + 1) * 128, :],
                                  in_=vm_sb[jc])

        # ---- tail: hg packed = h*g, then the packed y chunks only ----
        for oc in range(2):
            nc.vector.tensor_tensor(
                out=hg_sb[oc][:, 0:PJI], in0=h_sb[oc],
                in1=g_sb[oc][:, 0:PJI], op=mult)
        for oc in range(2):
            for off, w in chunks(PJI):
                emit_y(oc, off, w,
                       (nc.scalar, nc.sync) if oc == 0 else (nc.sync, nc.scalar))

    nc.compile()
    return nc


def _host_prep(x, mask, attn_bias, Wq, Wkv, Wout, Wg, bg, NJ, PJI):
    scale = DH ** -0.5
    PJ = NJ * 128
    NW = PJI + N

    def b16(a):
        return np.ascontiguousarray(a).astype(BF16)

    def dcpack(w):
        m = w.shape[1]
        return np.ascontiguousarray(
            w.reshape(2, 128, m).transpose(1, 0, 2).reshape(128, 2 * m))

    wq_p = b16(dcpack(Wq * (scale / TIE)))
    wk_p = b16(dcpack(Wkv[:, :INNER]))
    wv_p = b16(dcpack(Wkv[:, INNER:]))
    wg_p = b16(dcpack(Wg))
    wout_p = b16(dcpack(Wout))
    bg_p = np.ascontiguousarray(bg.reshape(2, 128).T).astype(np.float32)

    xsum_g = [x[g * TIE:(g + 1) * TIE].sum(0) for g in range(2)]  # [N, DIM]

    in_maps = []
    sels = []
    for c in range(NCORES):
        sel = np.where(mask[c])[0]
        n1 = len(sel)
        sels.append(sel)

        xp = np.zeros((DIM, PJ), np.float32)
        xp[:, :n1] = x[c, sel, :].T
        xs = np.zeros((DIM, PJI), np.float32)
        xs[:, :n1] = xsum_g[c // TIE][sel, :].T
        xo = np.zeros((DIM, NW), np.float32)
        xo[:, :n1] = x[c, sel, :].T
        xo[:, PJI:PJI + (N - n1)] = x[c, ~mask[c], :].T
        xsc = x[c].sum(0).reshape(2, 128).T  # [128, 2]

        eb = np.zeros((H * NJ * 128, PJI), np.float32)
        bias_c = attn_bias[0]                                # [H, N, N]
        for h in range(H):
            blk = np.exp(bias_c[h][np.ix_(sel, sel)].T)      # [j, i] packed
            eb[h * NJ * 128: h * NJ * 128 + n1, :n1] = blk

        in_maps.append({
            "xTp": b16(dcpack(xp)),
            "xsum": b16(dcpack(xs)),
            "xTo": b16(dcpack(xo)),
            "xsumc": b16(xsc),
            "ebp": np.ascontiguousarray(eb).astype(ml_dtypes.float8_e4m3fn),
            "wq": wq_p, "wk": wk_p, "wv": wv_p,
            "wg": wg_p, "wout": wout_p, "bg": bg_p,
        })
    return in_maps, sels


def kernel(x, mask, attn_bias, tie_dim, Wq, Wkv, Wout, bout, Wg, bg):
    global _compiled, _compiled_key, LAST_EXEC_NS, LAST_TRACE
    x = np.asarray(x, np.float32)
    mask_np = np.asarray(mask)
    attn_bias = np.asarray(attn_bias, np.float32)
    assert int(tie_dim) == TIE
    assert x.shape == (B, N, DIM) and mask_np.shape == (B, N)

    from concourse.bass_utils import run_bass_kernel_spmd

    n1s = mask_np.astype(np.int64).sum(axis=1)
    mx = int(n1s.max())
    NJ = max((mx + 127) // 128, 1)
    PJI = max(((mx + 31) // 32) * 32, 32)
    dbg = os.environ.get("KERNEL_DEBUG", "0")
    if _compiled is None or _compiled_key != (NJ, PJI, dbg):
        _compiled = _build(NJ, PJI)
        _compiled_key = (NJ, PJI, dbg)
    nc = _compiled

    in_maps, sels = _host_prep(
        x, mask_np, attn_bias,
        np.asarray(Wq, np.float32), np.asarray(Wkv, np.float32),
        np.asarray(Wout, np.float32), np.asarray(Wg, np.float32),
        np.asarray(bg, np.float32), NJ, PJI)

    trace = bool(int(os.environ.get("KERNEL_TRACE", "0")))
    res = run_bass_kernel_spmd(nc, in_maps, core_ids=list(range(NCORES)),
                               trace=trace)
    LAST_EXEC_NS = res.exec_time_ns
    LAST_TRACE = getattr(res, "profile_json", None)

    bout_f = np.asarray(bout, np.float32)
    y = np.empty((B, N, DIM), np.float32)
    for c in range(NCORES):
        o = np.asarray(res.results[c]["out"], np.float32)  # [256, NW]
        sel = sels[c]
        n1 = len(sel)
        y[c, sel, :] = o[:, :n1].T
        y[c, ~mask_np[c], :] = o[:, PJI:PJI + (N - n1)].T
    y += bout_f
    return y
